# revision 18
# baseline (speedup 1.0000x reference)
"""Trainium2 Bass kernel for nn_Net_18262200943034 (stereo cost-volume soft-argmin).

Math: out[b,h',w'] = soft-argmin over d' of trilinear-x4-upsampled
  vol[b,d,h,w] = [w>=d] * (SL[b,h,w] + SR[b,h,w-d]),  SL/SR = channel-means.

Key transformation (validated at ~6e-4 rel err vs reference, tol 2e-2):
softmax numerator N and denominator Z are LINEAR in e = exp(logit), so the
trilinear upsample is commuted past the exp (interp-of-exp instead of
exp-of-interp, second-order accurate).  Then per (h,w) knot:
  z[h,w] = eSL[h,w] * sum_d cz_d * eSR[h,w-d] + czcum[w]
  n[h,w] = eSL[h,w] * sum_d cn_d * eSR[h,w-d] + cncum[w]
with cz/cn = column sums of the D-upsample matrix (cn centered by -95.5*cz
for bf16 safety) and the shifted sum done as a CONVOLUTION along w = one
matmul against a constant 128x128 Toeplitz band.  z,n are H-upsampled, the
ratio is taken (reciprocal at [128,64], 8x cheaper than at full res), and
the W-upsample is applied to the centered ratio (interp-of-ratio, again
second-order).  +95.5 is restored by a rank-1 PSUM-accumulate matmul.

Per core (8 cores = batch 2 x four 64-row h' blocks):
  5 channel-sum MMs (fp8 data) -> exp[18,256] -> PE transpose -> Toeplitz MM
  -> *eSL, +tails -> H-up MM -> 2 transposes -> reciprocal/mul [128,64]
  -> W-up MM + MU rank-1 accum -> scalar-engine copy -> 2-queue DMA out.
Inputs travel as contiguous per-stream DRAM params to keep DMA descriptors
large; fp8 halves the big data block.
"""
import numpy as np
import ml_dtypes

import concourse.bacc as bacc
import concourse.bass as bass
import concourse.mybir as mybir
import concourse.tile as tile
from concourse.bass_utils import run_bass_kernel_spmd

F32 = mybir.dt.float32
BF16 = mybir.dt.bfloat16
FP8 = mybir.dt.float8e4
NPBF = ml_dtypes.bfloat16
NPF8 = ml_dtypes.float8_e4m3

B, C, H, W = 2, 32, 64, 128
D, DP = 48, 192
H4, W4 = 256, 512
HB = 64            # h' rows per core
HS = 18            # source h rows needed
HPAD = 20          # padded so C*HPAD = 640 = 5*128
KCH = 5
H_START = [0, 15, 31, 47]
MU = 95.5          # disparity centering constant (exact in bf16)

# data params fp8: dataS [128, 90] selector; dataR/dataL [128, 640] chunks
SEL_F = 90
# consA param [128, 256] bf16: Tz|Tn
# consB param [128, 512] bf16: Vw
# smal param [18, 320] bf16: Ah (64) | ccumT 4 rows (128) | tail-sel 4 rows (128)
OFF_CCUM = 64
OFF_TSEL = 192
SMAL_F = 320


def _interp_matrix(n_in, n_out):
    src = np.arange(n_out, dtype=np.float64) * ((n_in - 1) / (n_out - 1))
    i0 = np.clip(np.floor(src).astype(np.int64), 0, n_in - 1)
    i1 = np.clip(i0 + 1, 0, n_in - 1)
    w = src - i0
    M = np.zeros((n_in, n_out))
    for o in range(n_out):
        M[i0[o], o] += 1.0 - w[o]
        M[i1[o], o] += w[o]
    return M


def _shared():
    U = _interp_matrix(D, DP)
    cz = U.sum(1)
    cnc = (U * np.arange(DP)).sum(1) - MU * cz
    Tzn = np.zeros((128, 256), np.float32)
    for u in range(W):
        d = np.arange(min(D, W - u))
        Tzn[u, u + d] = cz[d]
        Tzn[u, W + u + d] = cnc[d]
    consA = Tzn.astype(NPBF)
    consB = _interp_matrix(W, W4).astype(np.float32).astype(NPF8)

    smal = np.zeros((HS, SMAL_F), np.float32)
    czc = np.array([cz[w + 1:].sum() for w in range(W)])
    cnn = np.array([cnc[w + 1:].sum() for w in range(W)])
    for r, v in ((0, czc), (2, cnn)):
        hi = v.astype(NPBF).astype(np.float64)
        smal[r, OFF_CCUM:OFF_CCUM + W] = hi
        smal[r + 1, OFF_CCUM:OFF_CCUM + W] = v - hi
    smal[0:2, OFF_TSEL:OFF_TSEL + HB] = 1.0
    smal[2:4, OFF_TSEL + HB:OFF_TSEL + 2 * HB] = 1.0

    sel = np.zeros((C * HPAD, HS), np.float32)
    for c in range(C):
        for h in range(HS):
            sel[c * HPAD + h, h] = 1.0 / (2 * C)
    selp = (sel.reshape(KCH, 128, HS).transpose(1, 0, 2)
            .reshape(128, KCH * HS).astype(NPF8))

    Ahf = _interp_matrix(H, H4)
    smals = []
    for j in range(4):
        hs = H_START[j]
        s = smal.copy()
        s[:min(H, hs + HS) - hs, 0:HB] = (
            Ahf[hs:min(H, hs + HS), HB * j:HB * (j + 1)])
        smals.append(s.astype(NPBF))
    return selp, consA, consB, smals


def _core_data(left, right, b, j):
    hs = H_START[j]
    nv = min(H, hs + HS) - hs
    lp = np.zeros((C, HPAD, W), np.float32)
    rp = np.zeros((C, HPAD, W), np.float32)
    lp[:, :nv] = left[b, :, hs:hs + nv]
    rp[:, :nv] = right[b, :, hs:hs + nv]
    dl = lp.reshape(KCH, 128, W).transpose(1, 0, 2).reshape(128, KCH * W)
    dr = rp.reshape(KCH, 128, W).transpose(1, 0, 2).reshape(128, KCH * W)
    return dl.astype(NPF8), dr.astype(NPF8)


def build_nc():
    nc = bacc.Bacc("TRN2", target_bir_lowering=False, debug=False)

    dataSR_d = nc.declare_dram_parameter("dataSR", [128, SEL_F + KCH * W], FP8,
                                          isOutput=False)
    dataL_d = nc.declare_dram_parameter("dataL", [128, KCH * W], FP8, isOutput=False)
    consA_d = nc.declare_dram_parameter("consA", [128, 256], BF16, isOutput=False)
    consB_d = nc.declare_dram_parameter("consB", [128, W4], FP8, isOutput=False)
    smal_d = nc.declare_dram_parameter("smal", [HS, SMAL_F], BF16, isOutput=False)
    outt_d = nc.declare_dram_parameter("outt", [32, 2 * W4], FP8, isOutput=True)

    EXP = mybir.ActivationFunctionType.Exp
    CPY = mybir.ActivationFunctionType.Copy

    with tile.TileContext(nc) as tc:
        with tc.tile_pool(name="sb", bufs=1) as pool:
            dataSR_sb = pool.tile([128, SEL_F + KCH * W], FP8)
            dataL_sb = pool.tile([128, KCH * W], FP8)
            consA_sb = pool.tile([128, 256], BF16)
            consB_sb = pool.tile([128, W4], FP8)
            smal_sb = pool.tile([HS, SMAL_F], BF16)
            nc.sync.dma_start(dataSR_sb[:], dataSR_d[:])
            nc.scalar.dma_start(dataL_sb[:], dataL_d[:])
            nc.gpsimd.dma_start(smal_sb[:], smal_d[:])
            nc.sync.dma_start(consA_sb[:], consA_d[:])
            nc.scalar.dma_start(consB_sb[:], consB_d[:])
            dataS_sb = dataSR_sb
            dataR_v = dataSR_sb[:, SEL_F:SEL_F + KCH * W]

            with tc.tile_pool(name="ps", bufs=1, space="PSUM") as ps:
                # SR^T [128 u, 18 h] directly: data chunks as stationary
                srt_ps = ps.tile([128, HS], F32)
                for k in range(KCH):
                    nc.tensor.matmul(
                        srt_ps[:], dataR_v[:, W * k:W * (k + 1)],
                        dataS_sb[:, HS * k:HS * (k + 1)],
                        start=(k == 0), stop=(k == KCH - 1))
                e_ut = pool.tile([128, HS], BF16)
                nc.scalar.activation(e_ut[:], srt_ps[:], EXP)

                # SL [18 h, 128 w]: selector as stationary
                sl_ps = ps.tile([HS, W], F32)
                for k in range(KCH):
                    nc.tensor.matmul(
                        sl_ps[:], dataS_sb[:, HS * k:HS * (k + 1)],
                        dataL_sb[:, W * k:W * (k + 1)],
                        start=(k == 0), stop=(k == KCH - 1))
                esl_sb = pool.tile([HS, W], BF16)
                nc.scalar.activation(esl_sb[:], sl_ps[:], EXP)

                # Toeplitz conv: Sz|Sn [18, 256]
                szn_ps = ps.tile([HS, 256], F32)
                nc.tensor.matmul(szn_ps[:], e_ut[:], consA_sb[:, 0:256],
                                 start=True, stop=True)

                # zl|nl = eSL * (Sz|Sn); ccum tails folded into H-up below
                zlnl = pool.tile([HS, 256], BF16)
                nc.vector.tensor_mul(
                    zlnl[:].rearrange("p (a w) -> p a w", a=2),
                    esl_sb[:].unsqueeze(1).broadcast_to((HS, 2, W)),
                    szn_ps[:].rearrange("p (a w) -> p a w", a=2))

                # H-up directly transposed: znT [128 w, 64 z | 64 n]
                # ccum-tail MM opens the group; zlnl halves accumulate in
                znt_ps = ps.tile([128, 128], F32)
                nc.tensor.matmul(znt_ps[:],
                                 smal_sb[0:4, OFF_CCUM:OFF_CCUM + W],
                                 smal_sb[0:4, OFF_TSEL:OFF_TSEL + 128],
                                 start=True, stop=False, skip_group_check=True)
                nc.tensor.matmul(znt_ps[:, 0:HB], zlnl[:, 0:W],
                                 smal_sb[:, 0:HB], start=False, stop=False,
                                 skip_group_check=True)
                nc.tensor.matmul(znt_ps[:, HB:128], zlnl[:, W:2 * W],
                                 smal_sb[:, 0:HB], start=False, stop=True,
                                 skip_group_check=True)

                # ratio on [128 w, 64 h'], split even/odd h' for pair-packing
                rzt = pool.tile([128, HB], F32)
                nc.vector.reciprocal_approx_fast(rzt[:], znt_ps[:, 0:HB])
                nv = znt_ps[:, HB:128].rearrange("p (h2 s) -> p h2 s", s=2)
                rv = rzt[:].rearrange("p (h2 s) -> p h2 s", s=2)
                oct_e = pool.tile([128, 32], FP8)
                oct_o = pool.tile([128, 32], FP8)
                nc.vector.tensor_mul(oct_e[:], nv[:, :, 0], rv[:, :, 0])
                nc.vector.tensor_mul(oct_o[:], nv[:, :, 1], rv[:, :, 1])

                # W-up, pair-packed output [32, 1024] for 2KB DMA rows
                out_ps = ps.tile([32, 2, W4], F32)
                nc.tensor.matmul(out_ps[:, 0, :], oct_e[:], consB_sb[:],
                                 start=True, stop=True)
                nc.tensor.matmul(out_ps[:, 1, :], oct_o[:], consB_sb[:],
                                 start=True, stop=True)

                outt_sb = pool.tile([32, 2 * W4], FP8)
                nc.vector.tensor_copy(
                    outt_sb[:].rearrange("p (s w) -> p s w", s=2), out_ps[:])
                nc.sync.dma_start(outt_d[0:16, :], outt_sb[0:16, :],
                                  single_packet=True)
                nc.gpsimd.dma_start(outt_d[16:32, :], outt_sb[16:32, :],
                                    single_packet=True)
    nc.compile()
    return nc


_NC = None
_SHARED = None


def _in_maps(left, right):
    global _SHARED
    if _SHARED is None:
        _SHARED = _shared()
    selp, consA, consB, smals = _SHARED
    maps = []
    for k in range(8):
        dl, dr = _core_data(left, right, k // 4, k % 4)
        maps.append({"dataSR": np.concatenate([selp, dr], axis=1),
                     "dataL": dl,
                     "consA": consA, "consB": consB, "smal": smals[k % 4]})
    return maps


def kernel(left, right):
    global _NC
    left = np.asarray(left, dtype=np.float32)
    right = np.asarray(right, dtype=np.float32)
    if _NC is None:
        _NC = build_nc()

    res = run_bass_kernel_spmd(_NC, _in_maps(left, right), core_ids=list(range(8)))
    out = np.zeros((B, H4, W4), np.float32)
    for k in range(8):
        b, j = k // 4, k % 4
        r = res.results[k]["outt"].astype(np.float32).reshape(HB, W4)
        out[b, HB * j:HB * (j + 1)] = r + np.float32(MU)
    return out


# revision 20
# speedup vs baseline: 1.0918x; 1.0918x over previous
"""Trainium2 Bass kernel for nn_Net_18262200943034 (stereo cost-volume soft-argmin).

Math: out[b,h',w'] = soft-argmin over d' of trilinear-x4-upsampled
  vol[b,d,h,w] = [w>=d] * (SL[b,h,w] + SR[b,h,w-d]),  SL/SR = channel-means.

Key transformation (measured 8.7e-4 rel err vs reference, tol 2e-2):
softmax numerator N and denominator Z are LINEAR in e = exp(logit), so the
trilinear upsample is commuted past the exp (interp-of-exp instead of
exp-of-interp, second-order accurate).  Then per (h,w) knot:
  z[h,w] = eSL[h,w] * sum_d cz_d * eSR[h,w-d] + czcum[w]
  n[h,w] = eSL[h,w] * sum_d cn_d * eSR[h,w-d] + cncum[w]
with cz/cn = column sums of the D-upsample matrix (cn centered by -95.5*cz
so every staged tensor is small; +95.5 is restored on the host) and the
shifted sum done as a CONVOLUTION along w = one matmul against a constant
128x128 Toeplitz band.  The H-upsample matmul emits directly in transposed
orientation (zl/nl halves as the stationary operand, Ah as moving), the
masked-region tails enter as a k=4 hi/lo rank-2 PSUM accumulation, the
ratio is a fast-approx reciprocal at [128,64], and the W-upsample applies
to the centered ratio (interp-of-ratio, second-order again).

Per core (8 cores = batch 2 x four 64-row h' blocks), ~30 instructions:
  channel-sum MMs (fp8 data; SR^T computed directly via data-as-weights)
  -> two tiny exps -> Toeplitz conv MM -> *eSL -> transposed H-up MMs
  -> approx-reciprocal / mul -> fp8 W-up MMs pair-packed to [32,1024]
  -> cast -> 2-queue DMA out (fp8 centered; host adds 95.5 and casts).
Inputs travel as contiguous per-stream DRAM params split over 3 DMA queues
(data fp8, interp consts fp8/bf16) to balance arrival against first use.
"""
import numpy as np
import ml_dtypes

import concourse.bacc as bacc
import concourse.bass as bass
import concourse.mybir as mybir
import concourse.tile as tile
from concourse.bass_utils import run_bass_kernel_spmd

F32 = mybir.dt.float32
BF16 = mybir.dt.bfloat16
FP8 = mybir.dt.float8e4
NPBF = ml_dtypes.bfloat16
NPF8 = ml_dtypes.float8_e4m3

B, C, H, W = 2, 32, 64, 128
D, DP = 48, 192
H4, W4 = 256, 512
HB = 64            # h' rows per core
HS = 18            # source h rows needed
HPAD = 20          # padded so C*HPAD = 640 = 5*128
KCH = 5
H_START = [0, 15, 31, 47]
MU = 95.5          # disparity centering constant (exact in bf16)

# data params fp8: dataS [128, 90] selector; dataR/dataL [128, 640] chunks
SEL_F = 90
# consA param [128, 256] bf16: Tz|Tn
# consB param [128, 512] bf16: Vw
# smal param [18, 320] bf16: Ah (64) | ccumT 4 rows (128) | tail-sel 4 rows (128)
OFF_CCUM = 64
OFF_TSEL = 192
SMAL_F = 320


def _interp_matrix(n_in, n_out):
    src = np.arange(n_out, dtype=np.float64) * ((n_in - 1) / (n_out - 1))
    i0 = np.clip(np.floor(src).astype(np.int64), 0, n_in - 1)
    i1 = np.clip(i0 + 1, 0, n_in - 1)
    w = src - i0
    M = np.zeros((n_in, n_out))
    for o in range(n_out):
        M[i0[o], o] += 1.0 - w[o]
        M[i1[o], o] += w[o]
    return M


def _shared():
    U = _interp_matrix(D, DP)
    cz = U.sum(1)
    cnc = (U * np.arange(DP)).sum(1) - MU * cz
    Tzn = np.zeros((128, 256), np.float32)
    for u in range(W):
        d = np.arange(min(D, W - u))
        Tzn[u, u + d] = cz[d]
        Tzn[u, W + u + d] = cnc[d]
    consA = Tzn.astype(NPBF)
    consB = _interp_matrix(W, W4).astype(np.float32).astype(NPF8)

    smal = np.zeros((HS, SMAL_F), np.float32)
    czc = np.array([cz[w + 1:].sum() for w in range(W)])
    cnn = np.array([cnc[w + 1:].sum() for w in range(W)])
    for r, v in ((0, czc), (2, cnn)):
        hi = v.astype(NPBF).astype(np.float64)
        smal[r, OFF_CCUM:OFF_CCUM + W] = hi
        smal[r + 1, OFF_CCUM:OFF_CCUM + W] = v - hi
    smal[0:2, OFF_TSEL:OFF_TSEL + HB] = 1.0
    smal[2:4, OFF_TSEL + HB:OFF_TSEL + 2 * HB] = 1.0

    sel = np.zeros((C * HPAD, HS), np.float32)
    for c in range(C):
        for h in range(HS):
            sel[c * HPAD + h, h] = 1.0 / (2 * C)
    selp = (sel.reshape(KCH, 128, HS).transpose(1, 0, 2)
            .reshape(128, KCH * HS).astype(NPF8))

    Ahf = _interp_matrix(H, H4)
    smals = []
    for j in range(4):
        hs = H_START[j]
        s = smal.copy()
        s[:min(H, hs + HS) - hs, 0:HB] = (
            Ahf[hs:min(H, hs + HS), HB * j:HB * (j + 1)])
        smals.append(s.astype(NPBF))
    return selp, consA, consB, smals


def _core_data(left, right, b, j):
    hs = H_START[j]
    nv = min(H, hs + HS) - hs
    lp = np.zeros((C, HPAD, W), np.float32)
    rp = np.zeros((C, HPAD, W), np.float32)
    lp[:, :nv] = left[b, :, hs:hs + nv]
    rp[:, :nv] = right[b, :, hs:hs + nv]
    dl = lp.reshape(KCH, 128, W).transpose(1, 0, 2).reshape(128, KCH * W)
    dr = rp.reshape(KCH, 128, W).transpose(1, 0, 2).reshape(128, KCH * W)
    return dl.astype(NPF8), dr.astype(NPF8)


def build_nc():
    nc = bacc.Bacc("TRN2", target_bir_lowering=False, debug=False)

    dataSR_d = nc.declare_dram_parameter("dataSR", [128, SEL_F + KCH * W], FP8,
                                          isOutput=False)
    dataL_d = nc.declare_dram_parameter("dataL", [128, KCH * W], FP8, isOutput=False)
    consA_d = nc.declare_dram_parameter("consA", [128, 256], BF16, isOutput=False)
    consB_d = nc.declare_dram_parameter("consB", [128, W4], FP8, isOutput=False)
    smal_d = nc.declare_dram_parameter("smal", [HS, SMAL_F], BF16, isOutput=False)
    outt_d = nc.declare_dram_parameter("outt", [32, 2 * W4], FP8, isOutput=True)

    EXP = mybir.ActivationFunctionType.Exp

    with tile.TileContext(nc) as tc:
        with tc.tile_pool(name="sb", bufs=1) as pool:
            dataSR_sb = pool.tile([128, SEL_F + KCH * W], FP8)
            dataL_sb = pool.tile([128, KCH * W], FP8)
            consA_sb = pool.tile([128, 256], BF16)
            consB_sb = pool.tile([128, W4], FP8)
            smal_sb = pool.tile([HS, SMAL_F], BF16)
            nc.sync.dma_start(dataSR_sb[:], dataSR_d[:])
            nc.scalar.dma_start(dataL_sb[:], dataL_d[:])
            nc.gpsimd.dma_start(smal_sb[:], smal_d[:])
            nc.sync.dma_start(consA_sb[:], consA_d[:])
            nc.scalar.dma_start(consB_sb[:], consB_d[:])
            dataS_sb = dataSR_sb
            dataR_v = dataSR_sb[:, SEL_F:SEL_F + KCH * W]

            with tc.tile_pool(name="ps", bufs=1, space="PSUM") as ps:
                # SR^T [128 u, 18 h] directly: data chunks as stationary
                srt_ps = ps.tile([128, HS], F32)
                for k in range(KCH):
                    nc.tensor.matmul(
                        srt_ps[:], dataR_v[:, W * k:W * (k + 1)],
                        dataS_sb[:, HS * k:HS * (k + 1)],
                        start=(k == 0), stop=(k == KCH - 1))
                e_ut = pool.tile([128, HS], BF16)
                nc.scalar.activation(e_ut[:], srt_ps[:], EXP)

                # SL [18 h, 128 w]: selector as stationary
                sl_ps = ps.tile([HS, W], F32)
                for k in range(KCH):
                    nc.tensor.matmul(
                        sl_ps[:], dataS_sb[:, HS * k:HS * (k + 1)],
                        dataL_sb[:, W * k:W * (k + 1)],
                        start=(k == 0), stop=(k == KCH - 1))
                esl_sb = pool.tile([HS, W], BF16)
                nc.scalar.activation(esl_sb[:], sl_ps[:], EXP)

                # Toeplitz conv: Sz|Sn [18, 256]
                szn_ps = ps.tile([HS, 256], F32)
                nc.tensor.matmul(szn_ps[:], e_ut[:], consA_sb[:, 0:256],
                                 start=True, stop=True)

                # zl|nl = eSL * (Sz|Sn); ccum tails folded into H-up below
                zlnl = pool.tile([HS, 256], BF16)
                nc.vector.tensor_mul(
                    zlnl[:].rearrange("p (a w) -> p a w", a=2),
                    esl_sb[:].unsqueeze(1).broadcast_to((HS, 2, W)),
                    szn_ps[:].rearrange("p (a w) -> p a w", a=2))

                # H-up directly transposed: znT [128 w, 64 z | 64 n]
                # ccum-tail MM opens the group; zlnl halves accumulate in
                znt_ps = ps.tile([128, 128], F32)
                nc.tensor.matmul(znt_ps[:],
                                 smal_sb[0:4, OFF_CCUM:OFF_CCUM + W],
                                 smal_sb[0:4, OFF_TSEL:OFF_TSEL + 128],
                                 start=True, stop=False, skip_group_check=True)
                nc.tensor.matmul(znt_ps[:, 0:HB], zlnl[:, 0:W],
                                 smal_sb[:, 0:HB], start=False, stop=False,
                                 skip_group_check=True)
                nc.tensor.matmul(znt_ps[:, HB:128], zlnl[:, W:2 * W],
                                 smal_sb[:, 0:HB], start=False, stop=True,
                                 skip_group_check=True)

                # ratio on [128 w, 64 h'], split even/odd h' for pair-packing
                rzt = pool.tile([128, HB], F32)
                nc.vector.reciprocal_approx_fast(rzt[:], znt_ps[:, 0:HB])
                nv = znt_ps[:, HB:128].rearrange("p (h2 s) -> p h2 s", s=2)
                rv = rzt[:].rearrange("p (h2 s) -> p h2 s", s=2)
                oct_e = pool.tile([128, 32], FP8)
                oct_o = pool.tile([128, 32], FP8)
                nc.vector.tensor_mul(oct_e[:], nv[:, :, 0], rv[:, :, 0])
                nc.vector.tensor_mul(oct_o[:], nv[:, :, 1], rv[:, :, 1])

                # W-up, pair-packed output [32, 1024] for 2KB DMA rows
                out_ps = ps.tile([32, 2, W4], F32)
                nc.tensor.matmul(out_ps[:, 0, :], oct_e[:], consB_sb[:],
                                 start=True, stop=True)
                nc.tensor.matmul(out_ps[:, 1, :], oct_o[:], consB_sb[:],
                                 start=True, stop=True)

                outt_sb = pool.tile([32, 2 * W4], FP8)
                nc.vector.tensor_copy(
                    outt_sb[:].rearrange("p (s w) -> p s w", s=2), out_ps[:])
                nc.sync.dma_start(outt_d[0:16, :], outt_sb[0:16, :])
                nc.gpsimd.dma_start(outt_d[16:32, :], outt_sb[16:32, :])
    nc.compile()
    return nc


_NC = None
_SHARED = None


def _in_maps(left, right):
    global _SHARED
    if _SHARED is None:
        _SHARED = _shared()
    selp, consA, consB, smals = _SHARED
    maps = []
    for k in range(8):
        dl, dr = _core_data(left, right, k // 4, k % 4)
        maps.append({"dataSR": np.concatenate([selp, dr], axis=1),
                     "dataL": dl,
                     "consA": consA, "consB": consB, "smal": smals[k % 4]})
    return maps


def kernel(left, right):
    global _NC
    left = np.asarray(left, dtype=np.float32)
    right = np.asarray(right, dtype=np.float32)
    if _NC is None:
        _NC = build_nc()

    res = run_bass_kernel_spmd(_NC, _in_maps(left, right), core_ids=list(range(8)))
    out = np.zeros((B, H4, W4), np.float32)
    for k in range(8):
        b, j = k // 4, k % 4
        r = res.results[k]["outt"].astype(np.float32).reshape(HB, W4)
        out[b, HB * j:HB * (j + 1)] = r + np.float32(MU)
    return out


# revision 21
# speedup vs baseline: 1.0960x; 1.0038x over previous
"""Trainium2 Bass kernel for nn_Net_18262200943034 (stereo cost-volume soft-argmin).

Math: out[b,h',w'] = soft-argmin over d' of trilinear-x4-upsampled
  vol[b,d,h,w] = [w>=d] * (SL[b,h,w] + SR[b,h,w-d]),  SL/SR = channel-means.

Key transformation (measured 8.7e-4 rel err vs reference, tol 2e-2):
softmax numerator N and denominator Z are LINEAR in e = exp(logit), so the
trilinear upsample is commuted past the exp (interp-of-exp instead of
exp-of-interp, second-order accurate).  Then per (h,w) knot:
  z[h,w] = eSL[h,w] * sum_d cz_d * eSR[h,w-d] + czcum[w]
  n[h,w] = eSL[h,w] * sum_d cn_d * eSR[h,w-d] + cncum[w]
with cz/cn = column sums of the D-upsample matrix (cn centered by -95.5*cz
so every staged tensor is small; +95.5 is restored on the host) and the
shifted sum done as a CONVOLUTION along w = one matmul against a constant
128x128 Toeplitz band.  The H-upsample matmul emits directly in transposed
orientation (zl/nl halves as the stationary operand, Ah as moving), the
masked-region tails enter as a k=4 hi/lo rank-2 PSUM accumulation, the
ratio is a fast-approx reciprocal at [128,64], and the W-upsample applies
to the centered ratio (interp-of-ratio, second-order again).

Per core (8 cores = batch 2 x four 64-row h' blocks), ~30 instructions:
  channel-sum MMs (fp8 data; SR^T computed directly via data-as-weights)
  -> two tiny exps -> Toeplitz conv MM -> *eSL -> transposed H-up MMs
  -> approx-reciprocal / mul -> fp8 W-up MMs pair-packed to [32,1024]
  -> cast -> 2-queue DMA out (fp8 centered; host adds 95.5 and casts).
Inputs travel as contiguous per-stream DRAM params split over 3 DMA queues
(data fp8, interp consts fp8/bf16) to balance arrival against first use.
"""
import numpy as np
import ml_dtypes

import concourse.bacc as bacc
import concourse.mybir as mybir
import concourse.tile as tile
from concourse.bass_utils import run_bass_kernel_spmd

F32 = mybir.dt.float32
BF16 = mybir.dt.bfloat16
FP8 = mybir.dt.float8e4
NPBF = ml_dtypes.bfloat16
NPF8 = ml_dtypes.float8_e4m3

B, C, H, W = 2, 32, 64, 128
D, DP = 48, 192
H4, W4 = 256, 512
HB = 64            # h' rows per core
HS = 18            # source h rows needed
HPAD = 20          # padded so C*HPAD = 640 = 5*128
KCH = 5
H_START = [0, 15, 31, 47]
MU = 95.5          # disparity centering constant (exact in bf16)

# dataSR param [128, 90+640] fp8: selector | right chunks; dataL [128, 640] fp8
SEL_F = 90
# consA param [128, 256] bf16: Tz|Tn
# consB param [128, 512] fp8: Vw
# smal param [18, 320] bf16: Ah (64) | ccumT 4 rows (128) | tail-sel 4 rows (128)
OFF_CCUM = 64
OFF_TSEL = 192
SMAL_F = 320


def _interp_matrix(n_in, n_out):
    src = np.arange(n_out, dtype=np.float64) * ((n_in - 1) / (n_out - 1))
    i0 = np.clip(np.floor(src).astype(np.int64), 0, n_in - 1)
    i1 = np.clip(i0 + 1, 0, n_in - 1)
    w = src - i0
    M = np.zeros((n_in, n_out))
    for o in range(n_out):
        M[i0[o], o] += 1.0 - w[o]
        M[i1[o], o] += w[o]
    return M


def _shared():
    U = _interp_matrix(D, DP)
    cz = U.sum(1)
    cnc = (U * np.arange(DP)).sum(1) - MU * cz
    Tzn = np.zeros((128, 256), np.float32)
    for u in range(W):
        d = np.arange(min(D, W - u))
        Tzn[u, u + d] = cz[d]
        Tzn[u, W + u + d] = cnc[d]
    consA = Tzn.astype(NPBF)
    consB = _interp_matrix(W, W4).astype(np.float32).astype(NPF8)

    smal = np.zeros((HS, SMAL_F), np.float32)
    czc = np.array([cz[w + 1:].sum() for w in range(W)])
    cnn = np.array([cnc[w + 1:].sum() for w in range(W)])
    for r, v in ((0, czc), (2, cnn)):
        hi = v.astype(NPBF).astype(np.float64)
        smal[r, OFF_CCUM:OFF_CCUM + W] = hi
        smal[r + 1, OFF_CCUM:OFF_CCUM + W] = v - hi
    smal[0:2, OFF_TSEL:OFF_TSEL + HB] = 1.0
    smal[2:4, OFF_TSEL + HB:OFF_TSEL + 2 * HB] = 1.0

    sel = np.zeros((C * HPAD, HS), np.float32)
    for c in range(C):
        for h in range(HS):
            sel[c * HPAD + h, h] = 1.0 / (2 * C)
    selp = (sel.reshape(KCH, 128, HS).transpose(1, 0, 2)
            .reshape(128, KCH * HS).astype(NPF8))

    Ahf = _interp_matrix(H, H4)
    smals = []
    for j in range(4):
        hs = H_START[j]
        s = smal.copy()
        s[:min(H, hs + HS) - hs, 0:HB] = (
            Ahf[hs:min(H, hs + HS), HB * j:HB * (j + 1)])
        smals.append(s.astype(NPBF))
    return selp, consA, consB, smals


def _core_data(left, right, b, j):
    hs = H_START[j]
    nv = min(H, hs + HS) - hs
    lp = np.zeros((C, HPAD, W), np.float32)
    rp = np.zeros((C, HPAD, W), np.float32)
    lp[:, :nv] = left[b, :, hs:hs + nv]
    rp[:, :nv] = right[b, :, hs:hs + nv]
    dl = lp.reshape(KCH, 128, W).transpose(1, 0, 2).reshape(128, KCH * W)
    dr = rp.reshape(KCH, 128, W).transpose(1, 0, 2).reshape(128, KCH * W)
    return dl.astype(NPF8), dr.astype(NPF8)


def build_nc():
    nc = bacc.Bacc("TRN2", target_bir_lowering=False, debug=False)

    dataSR_d = nc.declare_dram_parameter("dataSR", [128, SEL_F + KCH * W], FP8,
                                          isOutput=False)
    dataL_d = nc.declare_dram_parameter("dataL", [128, KCH * W], FP8, isOutput=False)
    consA_d = nc.declare_dram_parameter("consA", [128, 256], BF16, isOutput=False)
    consB_d = nc.declare_dram_parameter("consB", [128, W4], FP8, isOutput=False)
    smal_d = nc.declare_dram_parameter("smal", [HS, SMAL_F], BF16, isOutput=False)
    outt_d = nc.declare_dram_parameter("outt", [32, 2 * W4], FP8, isOutput=True)

    EXP = mybir.ActivationFunctionType.Exp

    with tile.TileContext(nc) as tc:
        with tc.tile_pool(name="sb", bufs=1) as pool:
            dataSR_sb = pool.tile([128, SEL_F + KCH * W], FP8)
            dataL_sb = pool.tile([128, KCH * W], FP8)
            consA_sb = pool.tile([128, 256], BF16)
            consB_sb = pool.tile([128, W4], FP8)
            smal_sb = pool.tile([HS, SMAL_F], BF16)
            nc.sync.dma_start(dataSR_sb[:], dataSR_d[:])
            nc.scalar.dma_start(dataL_sb[:], dataL_d[:])
            nc.gpsimd.dma_start(smal_sb[:], smal_d[:])
            nc.sync.dma_start(consA_sb[:], consA_d[:])
            nc.scalar.dma_start(consB_sb[:], consB_d[:])
            dataS_sb = dataSR_sb
            dataR_v = dataSR_sb[:, SEL_F:SEL_F + KCH * W]

            with tc.tile_pool(name="ps", bufs=1, space="PSUM") as ps:
                # SR^T [128 u, 18 h] directly: data chunks as stationary
                srt_ps = ps.tile([128, HS], F32)
                for k in range(KCH):
                    nc.tensor.matmul(
                        srt_ps[:], dataR_v[:, W * k:W * (k + 1)],
                        dataS_sb[:, HS * k:HS * (k + 1)],
                        start=(k == 0), stop=(k == KCH - 1))
                e_ut = pool.tile([128, HS], BF16)
                nc.scalar.activation(e_ut[:], srt_ps[:], EXP)

                # SL [18 h, 128 w]: selector as stationary
                sl_ps = ps.tile([HS, W], F32)
                for k in range(KCH):
                    nc.tensor.matmul(
                        sl_ps[:], dataS_sb[:, HS * k:HS * (k + 1)],
                        dataL_sb[:, W * k:W * (k + 1)],
                        start=(k == 0), stop=(k == KCH - 1))
                esl_sb = pool.tile([HS, W], BF16)
                nc.scalar.activation(esl_sb[:], sl_ps[:], EXP)

                # Toeplitz conv: Sz|Sn [18, 256]
                szn_ps = ps.tile([HS, 256], F32)
                nc.tensor.matmul(szn_ps[:], e_ut[:], consA_sb[:, 0:256],
                                 start=True, stop=True)

                # zl|nl = eSL * (Sz|Sn); ccum tails folded into H-up below
                zlnl = pool.tile([HS, 256], BF16)
                nc.vector.tensor_mul(
                    zlnl[:].rearrange("p (a w) -> p a w", a=2),
                    esl_sb[:].unsqueeze(1).broadcast_to((HS, 2, W)),
                    szn_ps[:].rearrange("p (a w) -> p a w", a=2))

                # H-up directly transposed: znT [128 w, 64 z | 64 n]
                # ccum-tail MM opens the group; zlnl halves accumulate in
                znt_ps = ps.tile([128, 128], F32)
                nc.tensor.matmul(znt_ps[:],
                                 smal_sb[0:4, OFF_CCUM:OFF_CCUM + W],
                                 smal_sb[0:4, OFF_TSEL:OFF_TSEL + 128],
                                 start=True, stop=False, skip_group_check=True)
                nc.tensor.matmul(znt_ps[:, 0:HB], zlnl[:, 0:W],
                                 smal_sb[:, 0:HB], start=False, stop=False,
                                 skip_group_check=True)
                nc.tensor.matmul(znt_ps[:, HB:128], zlnl[:, W:2 * W],
                                 smal_sb[:, 0:HB], start=False, stop=True,
                                 skip_group_check=True)

                # ratio on [128 w, 64 h'], split even/odd h' for pair-packing
                rzt = pool.tile([128, HB], F32)
                nc.vector.reciprocal_approx_fast(rzt[:], znt_ps[:, 0:HB])
                nv = znt_ps[:, HB:128].rearrange("p (h2 s) -> p h2 s", s=2)
                rv = rzt[:].rearrange("p (h2 s) -> p h2 s", s=2)
                oct_e = pool.tile([128, 32], FP8)
                oct_o = pool.tile([128, 32], FP8)
                nc.vector.tensor_mul(oct_e[:], nv[:, :, 0], rv[:, :, 0])
                nc.vector.tensor_mul(oct_o[:], nv[:, :, 1], rv[:, :, 1])

                # W-up, pair-packed output [32, 1024] for 2KB DMA rows
                out_ps = ps.tile([32, 2, W4], F32)
                nc.tensor.matmul(out_ps[:, 0, :], oct_e[:], consB_sb[:],
                                 start=True, stop=True)
                nc.tensor.matmul(out_ps[:, 1, :], oct_o[:], consB_sb[:],
                                 start=True, stop=True)

                outt_sb = pool.tile([32, 2 * W4], FP8)
                nc.vector.tensor_copy(
                    outt_sb[:].rearrange("p (s w) -> p s w", s=2), out_ps[:])
                nc.sync.dma_start(outt_d[0:16, :], outt_sb[0:16, :])
                nc.gpsimd.dma_start(outt_d[16:32, :], outt_sb[16:32, :])
    nc.compile()
    return nc


_NC = None
_SHARED = None


def _in_maps(left, right):
    global _SHARED
    if _SHARED is None:
        _SHARED = _shared()
    selp, consA, consB, smals = _SHARED
    maps = []
    for k in range(8):
        dl, dr = _core_data(left, right, k // 4, k % 4)
        maps.append({"dataSR": np.concatenate([selp, dr], axis=1),
                     "dataL": dl,
                     "consA": consA, "consB": consB, "smal": smals[k % 4]})
    return maps


def kernel(left, right):
    global _NC
    left = np.asarray(left, dtype=np.float32)
    right = np.asarray(right, dtype=np.float32)
    if _NC is None:
        _NC = build_nc()

    res = run_bass_kernel_spmd(_NC, _in_maps(left, right), core_ids=list(range(8)))
    out = np.zeros((B, H4, W4), np.float32)
    for k in range(8):
        b, j = k // 4, k % 4
        r = res.results[k]["outt"].astype(np.float32).reshape(HB, W4)
        out[b, HB * j:HB * (j + 1)] = r + np.float32(MU)
    return out


# revision 22
# speedup vs baseline: 1.1169x; 1.0190x over previous
"""Trainium2 Bass kernel for nn_Net_18262200943034 (stereo cost-volume soft-argmin).

Math: out[b,h',w'] = soft-argmin over d' of trilinear-x4-upsampled
  vol[b,d,h,w] = [w>=d] * (SL[b,h,w] + SR[b,h,w-d]),  SL/SR = channel-means.

Key transformation (measured 8.7e-4 rel err vs reference, tol 2e-2):
softmax numerator N and denominator Z are LINEAR in e = exp(logit), so the
trilinear upsample is commuted past the exp (interp-of-exp instead of
exp-of-interp, second-order accurate).  Then per (h,w) knot:
  z[h,w] = eSL[h,w] * sum_d cz_d * eSR[h,w-d] + czcum[w]
  n[h,w] = eSL[h,w] * sum_d cn_d * eSR[h,w-d] + cncum[w]
with cz/cn = column sums of the D-upsample matrix (cn centered by -95.5*cz
so every staged tensor is small; +95.5 is restored on the host) and the
shifted sum done as a CONVOLUTION along w = one matmul against a constant
128x128 Toeplitz band.  The H-upsample matmul emits directly in transposed
orientation (zl/nl halves as the stationary operand, Ah as moving), the
masked-region tails enter as a k=4 hi/lo rank-2 PSUM accumulation, the
ratio is a fast-approx reciprocal at [128,64], and the W-upsample applies
to the centered ratio (interp-of-ratio, second-order again).

Per core (8 cores = batch 2 x four 64-row h' blocks), ~30 instructions:
  channel-sum MMs (fp8 data; SR^T computed directly via data-as-weights)
  -> two tiny exps -> Toeplitz conv MM -> *eSL -> transposed H-up MMs
  -> approx-reciprocal / mul -> fp8 W-up MMs pair-packed to [32,1024]
  -> cast -> 2-queue DMA out (fp8 centered; host adds 95.5 and casts).
Inputs travel as contiguous per-stream DRAM params split over 3 DMA queues
(data fp8, interp consts fp8/bf16) to balance arrival against first use.
"""
import numpy as np
import ml_dtypes

import concourse.bacc as bacc
import concourse.mybir as mybir
import concourse.tile as tile
from concourse.bass_utils import run_bass_kernel_spmd

F32 = mybir.dt.float32
BF16 = mybir.dt.bfloat16
FP8 = mybir.dt.float8e4
NPBF = ml_dtypes.bfloat16
NPF8 = ml_dtypes.float8_e4m3

B, C, H, W = 2, 32, 64, 128
D, DP = 48, 192
H4, W4 = 256, 512
HB = 64            # h' rows per core
HS = 18            # source h rows needed
HPAD = 20          # padded so C*HPAD = 640 = 5*128
KCH = 5
H_START = [0, 15, 31, 47]
MU = 95.5          # disparity centering constant (exact in bf16)

# dataSR param [128, 90+640] fp8: selector | right chunks; dataL [128, 640] fp8
SEL_F = 90
# consA param [128, 256] bf16: Tz|Tn
# consB param [128, 512] fp8: Vw
# smal param [18, 320] bf16: Ah (64) | ccumT 4 rows (128) | tail-sel 4 rows (128)
OFF_CCUM = 64
OFF_TSEL = 192
SMAL_F = 320


def _interp_matrix(n_in, n_out):
    src = np.arange(n_out, dtype=np.float64) * ((n_in - 1) / (n_out - 1))
    i0 = np.clip(np.floor(src).astype(np.int64), 0, n_in - 1)
    i1 = np.clip(i0 + 1, 0, n_in - 1)
    w = src - i0
    M = np.zeros((n_in, n_out))
    for o in range(n_out):
        M[i0[o], o] += 1.0 - w[o]
        M[i1[o], o] += w[o]
    return M


def _shared():
    U = _interp_matrix(D, DP)
    cz = U.sum(1)
    cnc = (U * np.arange(DP)).sum(1) - MU * cz
    Tzn = np.zeros((128, 256), np.float32)
    for u in range(W):
        d = np.arange(min(D, W - u))
        Tzn[u, u + d] = cz[d]
        Tzn[u, W + u + d] = cnc[d]
    consA = Tzn.astype(NPBF)
    consB = _interp_matrix(W, W4).astype(np.float32).astype(NPF8)

    smal = np.zeros((HS, SMAL_F), np.float32)
    czc = np.array([cz[w + 1:].sum() for w in range(W)])
    cnn = np.array([cnc[w + 1:].sum() for w in range(W)])
    for r, v in ((0, czc), (2, cnn)):
        hi = v.astype(NPBF).astype(np.float64)
        smal[r, OFF_CCUM:OFF_CCUM + W] = hi
        smal[r + 1, OFF_CCUM:OFF_CCUM + W] = v - hi
    smal[0:2, OFF_TSEL:OFF_TSEL + HB] = 1.0
    smal[2:4, OFF_TSEL + HB:OFF_TSEL + 2 * HB] = 1.0

    sel = np.zeros((C * HPAD, HS), np.float32)
    for c in range(C):
        for h in range(HS):
            sel[c * HPAD + h, h] = 1.0 / (2 * C)
    selp = (sel.reshape(KCH, 128, HS).transpose(1, 0, 2)
            .reshape(128, KCH * HS).astype(NPF8))

    Ahf = _interp_matrix(H, H4)
    smals = []
    for j in range(4):
        hs = H_START[j]
        s = smal.copy()
        s[:min(H, hs + HS) - hs, 0:HB] = (
            Ahf[hs:min(H, hs + HS), HB * j:HB * (j + 1)])
        smals.append(s.astype(NPBF))
    return selp, consA, consB, smals


def _core_data(left, right, b, j):
    hs = H_START[j]
    nv = min(H, hs + HS) - hs
    lp = np.zeros((C, HPAD, W), np.float32)
    rp = np.zeros((C, HPAD, W), np.float32)
    lp[:, :nv] = left[b, :, hs:hs + nv]
    rp[:, :nv] = right[b, :, hs:hs + nv]
    dl = lp.reshape(KCH, 128, W).transpose(1, 0, 2).reshape(128, KCH * W)
    dr = rp.reshape(KCH, 128, W).transpose(1, 0, 2).reshape(128, KCH * W)
    return dl.astype(NPF8), dr.astype(NPF8)


def build_nc():
    nc = bacc.Bacc("TRN2", target_bir_lowering=False, debug=False)

    dataSR_d = nc.declare_dram_parameter("dataSR", [128, SEL_F + KCH * W], FP8,
                                          isOutput=False)
    dataL_d = nc.declare_dram_parameter("dataL", [128, KCH * W], FP8, isOutput=False)
    consA_d = nc.declare_dram_parameter("consA", [128, 256], BF16, isOutput=False)
    consB_d = nc.declare_dram_parameter("consB", [128, W4], FP8, isOutput=False)
    smal_d = nc.declare_dram_parameter("smal", [HS, SMAL_F], BF16, isOutput=False)
    outt_d = nc.declare_dram_parameter("outt", [32, 2 * W4], FP8, isOutput=True)

    EXP = mybir.ActivationFunctionType.Exp

    with tile.TileContext(nc) as tc:
        with tc.tile_pool(name="sb", bufs=1) as pool:
            dataSR_sb = pool.tile([128, SEL_F + KCH * W], FP8)
            dataL_sb = pool.tile([128, KCH * W], FP8)
            consA_sb = pool.tile([128, 256], BF16)
            consB_sb = pool.tile([128, W4], FP8)
            smal_sb = pool.tile([HS, SMAL_F], BF16)
            nc.sync.dma_start(dataSR_sb[:], dataSR_d[:])
            nc.scalar.dma_start(dataL_sb[:], dataL_d[:])
            nc.gpsimd.dma_start(smal_sb[:], smal_d[:])
            nc.gpsimd.dma_start(consA_sb[:], consA_d[:])
            nc.scalar.dma_start(consB_sb[:], consB_d[:])
            dataS_sb = dataSR_sb
            dataR_v = dataSR_sb[:, SEL_F:SEL_F + KCH * W]

            with tc.tile_pool(name="ps", bufs=1, space="PSUM") as ps:
                # SR^T [128 u, 18 h] directly: data chunks as stationary
                srt_ps = ps.tile([128, HS], F32)
                for k in range(KCH):
                    nc.tensor.matmul(
                        srt_ps[:], dataR_v[:, W * k:W * (k + 1)],
                        dataS_sb[:, HS * k:HS * (k + 1)],
                        start=(k == 0), stop=(k == KCH - 1))
                e_ut = pool.tile([128, HS], BF16)
                nc.scalar.activation(e_ut[:], srt_ps[:], EXP)

                # SL [18 h, 128 w]: selector as stationary
                sl_ps = ps.tile([HS, W], F32)
                for k in range(KCH):
                    nc.tensor.matmul(
                        sl_ps[:], dataS_sb[:, HS * k:HS * (k + 1)],
                        dataL_sb[:, W * k:W * (k + 1)],
                        start=(k == 0), stop=(k == KCH - 1))
                esl_sb = pool.tile([HS, W], BF16)
                nc.scalar.activation(esl_sb[:], sl_ps[:], EXP)

                # Toeplitz conv: Sz|Sn [18, 256]
                szn_ps = ps.tile([HS, 256], F32)
                nc.tensor.matmul(szn_ps[:], e_ut[:], consA_sb[:, 0:256],
                                 start=True, stop=True)

                # zl|nl = eSL * (Sz|Sn); ccum tails folded into H-up below
                zlnl = pool.tile([HS, 256], BF16)
                nc.vector.tensor_mul(
                    zlnl[:].rearrange("p (a w) -> p a w", a=2),
                    esl_sb[:].unsqueeze(1).broadcast_to((HS, 2, W)),
                    szn_ps[:].rearrange("p (a w) -> p a w", a=2))

                # H-up directly transposed: znT [128 w, 64 z | 64 n]
                # ccum-tail MM opens the group; zlnl halves accumulate in
                znt_ps = ps.tile([128, 128], F32)
                nc.tensor.matmul(znt_ps[:],
                                 smal_sb[0:4, OFF_CCUM:OFF_CCUM + W],
                                 smal_sb[0:4, OFF_TSEL:OFF_TSEL + 128],
                                 start=True, stop=False, skip_group_check=True)
                nc.tensor.matmul(znt_ps[:, 0:HB], zlnl[:, 0:W],
                                 smal_sb[:, 0:HB], start=False, stop=False,
                                 skip_group_check=True)
                nc.tensor.matmul(znt_ps[:, HB:128], zlnl[:, W:2 * W],
                                 smal_sb[:, 0:HB], start=False, stop=True,
                                 skip_group_check=True)

                # ratio on [128 w, 64 h'], split even/odd h' for pair-packing
                rzt = pool.tile([128, HB], F32)
                nc.vector.reciprocal_approx_fast(rzt[:], znt_ps[:, 0:HB])
                nv = znt_ps[:, HB:128].rearrange("p (h2 s) -> p h2 s", s=2)
                rv = rzt[:].rearrange("p (h2 s) -> p h2 s", s=2)
                oct_e = pool.tile([128, 32], FP8)
                oct_o = pool.tile([128, 32], FP8)
                nc.vector.tensor_mul(oct_e[:], nv[:, :, 0], rv[:, :, 0])
                nc.vector.tensor_mul(oct_o[:], nv[:, :, 1], rv[:, :, 1])

                # W-up, pair-packed output [32, 1024] for 2KB DMA rows;
                # separate PSUM tiles so each cast chases its own matmul
                out_pe = ps.tile([32, W4], F32)
                out_po = ps.tile([32, W4], F32)
                nc.tensor.matmul(out_pe[:], oct_e[:], consB_sb[:],
                                 start=True, stop=True)
                nc.tensor.matmul(out_po[:], oct_o[:], consB_sb[:],
                                 start=True, stop=True)

                outt_sb = pool.tile([32, 2 * W4], FP8)
                ov = outt_sb[:].rearrange("p (s w) -> p s w", s=2)
                nc.vector.tensor_copy(ov[:, 0, :], out_pe[:])
                nc.vector.tensor_copy(ov[:, 1, :], out_po[:])
                nc.sync.dma_start(outt_d[0:16, :], outt_sb[0:16, :])
                nc.gpsimd.dma_start(outt_d[16:32, :], outt_sb[16:32, :])
    nc.compile()
    return nc


_NC = None
_SHARED = None


def _in_maps(left, right):
    global _SHARED
    if _SHARED is None:
        _SHARED = _shared()
    selp, consA, consB, smals = _SHARED
    maps = []
    for k in range(8):
        dl, dr = _core_data(left, right, k // 4, k % 4)
        maps.append({"dataSR": np.concatenate([selp, dr], axis=1),
                     "dataL": dl,
                     "consA": consA, "consB": consB, "smal": smals[k % 4]})
    return maps


def kernel(left, right):
    global _NC
    left = np.asarray(left, dtype=np.float32)
    right = np.asarray(right, dtype=np.float32)
    if _NC is None:
        _NC = build_nc()

    res = run_bass_kernel_spmd(_NC, _in_maps(left, right), core_ids=list(range(8)))
    out = np.zeros((B, H4, W4), np.float32)
    for k in range(8):
        b, j = k // 4, k % 4
        r = res.results[k]["outt"].astype(np.float32).reshape(HB, W4)
        out[b, HB * j:HB * (j + 1)] = r + np.float32(MU)
    return out
